# revision 10
# baseline (speedup 1.0000x reference)
"""Trainium2 Bass kernel for the autoregressive GRU-with-head problem.

Shapes: B=256, H=512, C=4096, L=48.
    codes0 = sigmoid(noise @ w_out.T + b_out)
    repeat L: gi = codes @ w_ih.T + b_ih ; gh = h @ w_hh.T + b_hh
              r,z = sigmoid(gi_rz + gh_rz) ; n = tanh(gi_n + r*gh_n)
              h = n + z*(h-n) ; codes = sigmoid(h @ w_out.T + b_out)
    outputs: samples[b,t,:] = codes BEFORE step t ; hiddens[b,t,:] = h AFTER step t

Strategy: pure data-parallel over batch (32 rows per core, 8 cores), weights
replicated and resident in SBUF as bf16 (fp32 doesn't fit in 24MB). All
matmuls are weight-stationary with the batch (N=32) streaming; accumulation
in fp32 PSUM. Biases are pre-loaded into PSUM via a small selection matmul
(start=True) so sigmoid/tanh read bias-included values straight from PSUM.

Layout convention: everything is transposed on-chip — feature dims live on
partitions (chunks of 128), batch is the free dim. A tensor X[dim, b] with
dim = k*128 + p is stored flat as sbuf[p, k*32 + b].
"""

import numpy as np
import ml_dtypes

BF16 = ml_dtypes.bfloat16

B, H, C, L = 256, 512, 4096, 48
NCORES = 8
BL = B // NCORES          # 32 batch rows per core
KC = C // 128             # 32 contraction chunks over C
KH = H // 128             # 4 contraction chunks over H
MG = (3 * H) // 128       # 12 gate m-tiles (r:0-3, z:4-7, n:8-11)
MC = C // 128             # 32 head m-tiles


def build(nc, tc, L_steps=L):
    """Emit the per-core program into TileContext tc. Returns nothing."""
    import concourse.bass as bass
    import concourse.mybir as mybir

    f32 = mybir.dt.float32
    bf16 = mybir.dt.bfloat16
    Sigmoid = mybir.ActivationFunctionType.Sigmoid
    Tanh = mybir.ActivationFunctionType.Tanh

    # ---- dram tensors ----
    w_ihT = nc.dram_tensor("w_ihT", [C, 3 * H], bf16, kind="ExternalInput").ap()
    w_hhT = nc.dram_tensor("w_hhT", [H, 3 * H], bf16, kind="ExternalInput").ap()
    w_outT = nc.dram_tensor("w_outT", [H, C], bf16, kind="ExternalInput").ap()
    bias_g = nc.dram_tensor("bias_g", [16, 128], bf16, kind="ExternalInput").ap()
    bias_o = nc.dram_tensor("bias_o", [32, 128], bf16, kind="ExternalInput").ap()
    sel_g = nc.dram_tensor("sel_g", [16, 512], bf16, kind="ExternalInput").ap()
    sel_o = nc.dram_tensor("sel_o", [32, 1024], bf16, kind="ExternalInput").ap()
    noiseT = nc.dram_tensor("noiseT", [H, BL], bf16, kind="ExternalInput").ap()
    # outputs are [t, feature, batch] per core — batch innermost so the DMA's
    # contiguous final dim lines up with the sbuf free dim; host transposes.
    samples = nc.dram_tensor("samples", [L, C, BL], f32, kind="ExternalOutput").ap()
    hiddens = nc.dram_tensor("hiddens", [L, H, BL], f32, kind="ExternalOutput").ap()

    # ---- pools ----
    from contextlib import ExitStack
    ctx = ExitStack()
    wpool = ctx.enter_context(tc.tile_pool(name="weights", bufs=1))
    spool = ctx.enter_context(tc.tile_pool(name="state", bufs=2))
    tpool = ctx.enter_context(tc.tile_pool(name="tmp", bufs=2))
    pg_pool = ctx.enter_context(tc.tile_pool(name="pg", bufs=2, space="PSUM"))
    pc_pool = ctx.enter_context(tc.tile_pool(name="pc", bufs=2, space="PSUM"))

    # ---- persistent sbuf tensors ----
    W_ih = wpool.tile([128, KC * 1536], bf16, tag="w_ih")     # [p, k*1536+f]
    W_hh = wpool.tile([128, KH * 1536], bf16, tag="w_hh")
    W_out = wpool.tile([128, KH * 4096], bf16, tag="w_out")
    BiasG = wpool.tile([128, 128], bf16, tag="bias_g")
    BiasO = wpool.tile([128, 128], bf16, tag="bias_o")
    SelG = wpool.tile([128, 512], bf16, tag="sel_g")
    SelO = wpool.tile([128, 1024], bf16, tag="sel_o")
    Noise = wpool.tile([128, KH * BL], bf16, tag="noise")
    H0 = wpool.tile([128, KH * BL], f32, tag="h0")

    # ---- prologue DMAs ----
    nc.sync.dma_start(Noise[:].rearrange("p (k b) -> p k b", b=BL),
                      noiseT.rearrange("(k p) b -> p k b", p=128))
    nc.sync.dma_start(BiasO[0:32, :], bias_o)
    nc.sync.dma_start(SelO[0:32, :], sel_o)
    nc.sync.dma_start(W_out[:].rearrange("p (k f) -> p k f", f=4096),
                      w_outT.rearrange("(k p) f -> p k f", p=128))
    nc.sync.dma_start(BiasG[0:16, :], bias_g)
    nc.sync.dma_start(SelG[0:16, :], sel_g)
    nc.sync.dma_start(W_hh[:].rearrange("p (k f) -> p k f", f=1536),
                      w_hhT.rearrange("(k p) f -> p k f", p=128))
    for k in range(KC):
        nc.sync.dma_start(W_ih[:, k * 1536:(k + 1) * 1536],
                          w_ihT[k * 128:(k + 1) * 128, :])
    nc.vector.memset(H0[:], 0.0)

    # warm the ACT table (sigmoid_and_others includes tanh) on a dummy so the
    # first real sigmoid doesn't carry the implicit table-load (walrus adds a
    # sync wait for it, overflowing the ACT wait-slot limit).
    warm = wpool.tile([128, 1], f32, tag="warm")
    nc.vector.memset(warm[:], 0.0)
    nc.scalar.activation(warm[:], warm[:], Sigmoid)
    nc.scalar.activation(warm[:], warm[:], Tanh)

    # stationary slice helpers: lhsT tile [K=128, M=128]
    ih = lambda k, m: W_ih[:, k * 1536 + m * 128: k * 1536 + (m + 1) * 128]
    hh = lambda k, m: W_hh[:, k * 1536 + m * 128: k * 1536 + (m + 1) * 128]
    ot = lambda k, m: W_out[:, k * 4096 + m * 128: k * 4096 + (m + 1) * 128]

    def head(rhs_b16, sfx):
        """codes_pre = w_out @ rhs + b_out -> sigmoid -> (f32, bf16) tiles.
        rhs_b16: [128, KH*BL] bf16. Returns (codes_f32, codes_b16), each
        [128, MC*BL] flat, produced as two halves for pipelining."""
        pc = pc_pool.tile([128, MC * BL], mybir.dt.float32, tag="pc")
        c_f32 = spool.tile([128, MC * BL], mybir.dt.float32, tag="codes_f32")
        c_b16 = spool.tile([128, MC * BL], bf16, tag="codes_b16")
        for half in range(2):
            cols = slice(half * 512, (half + 1) * 512)
            nc.tensor.matmul(pc[:, cols], BiasO[0:32, :], SelO[0:32, cols],
                             start=True, stop=False, skip_group_check=True)
            for m in range(half * 16, (half + 1) * 16):
                for k in range(KH):
                    nc.tensor.matmul(
                        pc[:, m * BL:(m + 1) * BL],
                        ot(k, m),
                        rhs_b16[:, k * BL:(k + 1) * BL],
                        start=False, stop=(k == KH - 1), skip_group_check=True)
            nc.scalar.activation(c_f32[:, cols], pc[:, cols], Sigmoid)
            nc.vector.tensor_copy(c_b16[:, cols], c_f32[:, cols])
        return c_f32, c_b16

    codes_f32, codes_b16 = head(Noise[:], "init")
    h_f32 = H0
    h_b16 = None  # t=0: h == 0, gh matmuls skipped

    for t in range(L_steps):
        # emit samples[t] = codes(t)
        nc.sync.dma_start(
            samples[t, :, :].rearrange("(m p) b -> p m b", p=128),
            codes_f32[:].rearrange("p (m b) -> p m b", b=BL))

        # ---- gates: psum bank layout [rz 0:256 | i_n 256:384 | h_n 384:512]
        pg = pg_pool.tile([128, 512], mybir.dt.float32, tag="pg")
        nc.tensor.matmul(pg[:], BiasG[0:16, :], SelG[0:16, :],
                         start=True, stop=False, skip_group_check=True)
        if h_b16 is not None:
            for m in range(8):      # rz: gh contribution
                for k in range(KH):
                    nc.tensor.matmul(
                        pg[:, m * BL:(m + 1) * BL], hh(k, m),
                        h_b16[:, k * BL:(k + 1) * BL],
                        start=False, stop=False, skip_group_check=True)
            for m in range(4):      # h_n
                for k in range(KH):
                    nc.tensor.matmul(
                        pg[:, 384 + m * BL: 384 + (m + 1) * BL], hh(k, 8 + m),
                        h_b16[:, k * BL:(k + 1) * BL],
                        start=False, stop=(k == KH - 1), skip_group_check=True)
        for m in range(8):          # rz: gi contribution
            for k in range(KC):
                nc.tensor.matmul(
                    pg[:, m * BL:(m + 1) * BL], ih(k, m),
                    codes_b16[:, k * BL:(k + 1) * BL],
                    start=False, stop=(k == KC - 1), skip_group_check=True)
        for m in range(4):          # i_n
            for k in range(KC):
                nc.tensor.matmul(
                    pg[:, 256 + m * BL: 256 + (m + 1) * BL], ih(k, 8 + m),
                    codes_b16[:, k * BL:(k + 1) * BL],
                    start=False, stop=(k == KC - 1), skip_group_check=True)

        # ---- gate math ----
        r_z = tpool.tile([128, 256], mybir.dt.float32, tag="r_z")
        nc.scalar.activation(r_z[:], pg[:, 0:256], Sigmoid)
        q = tpool.tile([128, 128], mybir.dt.float32, tag="q")
        nc.vector.tensor_mul(q[:], r_z[:, 0:128], pg[:, 384:512])   # r * h_n
        s = tpool.tile([128, 128], mybir.dt.float32, tag="s")
        nc.vector.tensor_add(s[:], q[:], pg[:, 256:384])            # + i_n
        n_t = tpool.tile([128, 128], mybir.dt.float32, tag="n")
        nc.scalar.activation(n_t[:], s[:], Tanh)
        d = tpool.tile([128, 128], mybir.dt.float32, tag="d")
        nc.vector.tensor_sub(d[:], h_f32[:], n_t[:])                # h - n
        p_t = tpool.tile([128, 128], mybir.dt.float32, tag="p")
        nc.vector.tensor_mul(p_t[:], r_z[:, 128:256], d[:])        # z*(h-n)
        h_new = spool.tile([128, KH * BL], mybir.dt.float32, tag="h_f32")
        nc.vector.tensor_add(h_new[:], n_t[:], p_t[:])
        h_newb = spool.tile([128, KH * BL], bf16, tag="h_b16")
        nc.vector.tensor_copy(h_newb[:], h_new[:])

        nc.sync.dma_start(
            hiddens[t, :, :].rearrange("(k p) b -> p k b", p=128),
            h_new[:].rearrange("p (k b) -> p k b", b=BL))

        h_f32, h_b16 = h_new, h_newb
        if t < L_steps - 1:
            codes_f32, codes_b16 = head(h_newb[:], f"t{t}")

    ctx.close()


def host_prep(noise, w_ih, b_ih, w_hh, b_hh, w_out, b_out):
    """Build the per-core input maps (numpy)."""
    f32 = np.float32
    w_ihT = np.ascontiguousarray(w_ih.astype(f32).T).astype(BF16)    # [C, 3H]
    w_hhT = np.ascontiguousarray(w_hh.astype(f32).T).astype(BF16)    # [H, 3H]
    w_outT = np.ascontiguousarray(w_out.astype(f32).T).astype(BF16)  # [H, C]

    bias_g = np.zeros((16, 128), f32)
    b_rz = (b_ih[:1024] + b_hh[:1024]).astype(f32)
    for m in range(8):
        bias_g[m] = b_rz[m * 128:(m + 1) * 128]
    for m in range(4):
        bias_g[8 + m] = b_ih[1024 + m * 128: 1024 + (m + 1) * 128]
        bias_g[12 + m] = b_hh[1024 + m * 128: 1024 + (m + 1) * 128]
    bias_o = b_out.astype(f32).reshape(32, 128)

    sel_g = np.zeros((16, 512), f32)
    for m in range(8):
        sel_g[m, m * 32:(m + 1) * 32] = 1.0
    for m in range(4):
        sel_g[8 + m, 256 + m * 32: 256 + (m + 1) * 32] = 1.0
        sel_g[12 + m, 384 + m * 32: 384 + (m + 1) * 32] = 1.0
    sel_o = np.zeros((32, 1024), f32)
    for m in range(32):
        sel_o[m, m * 32:(m + 1) * 32] = 1.0

    noiseT = np.ascontiguousarray(noise.astype(f32).T).astype(BF16)  # [H, B]

    common = dict(
        w_ihT=w_ihT, w_hhT=w_hhT, w_outT=w_outT,
        bias_g=bias_g.astype(BF16), bias_o=bias_o.astype(BF16),
        sel_g=sel_g.astype(BF16), sel_o=sel_o.astype(BF16),
    )
    in_maps = []
    for i in range(NCORES):
        m = dict(common)
        m["noiseT"] = np.ascontiguousarray(noiseT[:, i * BL:(i + 1) * BL])
        in_maps.append(m)
    return in_maps


_BUILD_CACHE = {}


def build_nc(L_steps=L):
    if L_steps in _BUILD_CACHE:
        return _BUILD_CACHE[L_steps]
    import concourse.bacc as bacc
    import concourse.tile as tile
    nc = bacc.Bacc("TRN2", target_bir_lowering=False, debug=False,
                   enable_asserts=False, num_devices=NCORES)
    with tile.TileContext(nc) as tc:
        build(nc, tc, L_steps)
    nc.compile()
    _BUILD_CACHE[L_steps] = nc
    return nc


def kernel(noise, w_ih, b_ih, w_hh, b_hh, w_out, b_out, max_len):
    assert int(max_len) == L, f"kernel hardcodes L={L}, got {max_len}"
    from concourse.bass_utils import run_bass_kernel_spmd

    noise = np.asarray(noise, np.float32)
    in_maps = host_prep(np.asarray(noise, np.float32),
                        np.asarray(w_ih, np.float32),
                        np.asarray(b_ih, np.float32),
                        np.asarray(w_hh, np.float32),
                        np.asarray(b_hh, np.float32),
                        np.asarray(w_out, np.float32),
                        np.asarray(b_out, np.float32))
    nc = build_nc(L)
    res = run_bass_kernel_spmd(nc, in_maps, core_ids=list(range(NCORES)))
    # per-core outputs are [L, C, BL]; reorder to [B, L, C]
    samples = np.concatenate(
        [np.transpose(r["samples"], (2, 0, 1)) for r in res.results], axis=0)
    hiddens = np.concatenate(
        [np.transpose(r["hiddens"], (2, 0, 1)) for r in res.results], axis=0)
    return np.ascontiguousarray(samples), np.ascontiguousarray(hiddens)


if __name__ == "__main__":
    nc = build_nc(2)
    print("built OK")
